# revision 1
# baseline (speedup 1.0000x reference)
"""Trainium2 Bass kernel for a basic RNN layer.

Reference: h_t = relu(concat([x_t, h_{t-1}]) @ W + b), outputs all h_t.
Shapes: x [64, 512, 1024], W [2048, 1024], b [1024]; out [64, 512, 1024] f32.

Data-parallel over batch (8 cores x 8 rows), hidden-major layout per
core: hidden on SBUF partitions, (time, batch) on the free dim.
U = x@W_x + b is precomputed at the bf16 PE roofline (~113us); the
recurrence then runs 64 LDWEIGHTS+MATMUL pairs per step at the
measured ~27ns/pair weight-load floor, u_t injected via identity-
weight matmuls, relu+cast epilogues on the DVE.

Step structure: two PSUM halves (m0-3, m4-7), bufs=8.  relu1 is split
into two DVE ops (h_new and a copy h_hi) and half 1's k-high matmuls
read the copy, so the two halves' k-high batches have genuinely
different readiness and the scheduler cannot bunch their stops at the
stream end.  Measured ~2.26us/step at the full 2.4GHz clock; total
~1.29ms.  Relative error vs fp32 reference ~3.6e-3.

Measured dead ends: fp8 weights (loads are row-rate-limited -> no
speedup), DoubleRow fp8 (accuracy), emission-order scheduling tricks
and tile_wait_until pins (the Tile scheduler reorders by its own
optimistic cost model), precompute/recurrence interleave (additive).
"""

import numpy as np
import ml_dtypes

import concourse.bass as bass
import concourse.bacc as bacc
import concourse.tile as tile
import concourse.mybir as mybir
from concourse.bass_utils import run_bass_kernel_spmd

BF16 = ml_dtypes.bfloat16

B, T, D, H = 64, 512, 1024, 1024
NCORES = 8
BC = B // NCORES        # batch rows per core = 8
KD = D // 128           # input-dim chunks = 8
KH = H // 128           # hidden-dim chunks = 8
MCH = H // 128          # output-hidden chunks = 8
SW = MCH * BC           # step width in free-dim columns = 64


def build_nc(t_steps: int = T):
    """Build the per-core Bass program (SPMD: all cores run this NEFF)."""
    nb = t_steps * BC  # total (t, b) columns
    nt = min(512, nb)  # moving-dim chunk for the U precompute
    assert nb % nt == 0

    f32 = mybir.dt.float32
    bf16 = mybir.dt.bfloat16

    nc = bacc.Bacc("TRN2", target_bir_lowering=False, debug=False)
    xT = nc.dram_tensor("xT", [128, KD * nb], bf16, kind="ExternalInput").ap()
    Wx = nc.dram_tensor("Wx", [128, KD * H], bf16, kind="ExternalInput").ap()
    Wh = nc.dram_tensor("Wh", [128, KH * H], bf16, kind="ExternalInput").ap()
    bias = nc.dram_tensor("bias", [128, MCH], f32, kind="ExternalInput").ap()
    ident = nc.dram_tensor("ident", [128, 128], bf16, kind="ExternalInput").ap()
    Y = nc.dram_tensor("Y", [t_steps, 128, SW], bf16, kind="ExternalOutput").ap()

    with tile.TileContext(nc) as tc, \
            tc.tile_pool(name="const", bufs=1) as const_pool, \
            tc.tile_pool(name="xin", bufs=3) as xpool, \
            tc.tile_pool(name="u", bufs=1) as upool, \
            tc.tile_pool(name="h", bufs=6) as hpool:

        wx_sb = const_pool.tile([128, KD * H], bf16, tag="wx")
        wh_sb = const_pool.tile([128, KH * H], bf16, tag="wh")
        b_sb = const_pool.tile([128, MCH], f32, tag="bias")
        id_sb = const_pool.tile([128, 128], bf16, tag="ident")
        u_sb = upool.tile([128, t_steps * SW], bf16)

        for k in range(KD):
            nc.sync.dma_start(wx_sb[:, k * H:(k + 1) * H], Wx[:, k * H:(k + 1) * H])
        nc.sync.dma_start(b_sb[:], bias[:])

        # ---- Precompute U.T = W_x.T @ x.T + b  (bf16 into SBUF) ----
        # u_sb column layout: t*SW + m*BC + b, matching the recurrence psum.
        uv = u_sb[:].rearrange("p (t m b) -> p t m b", m=MCH, b=BC)
        tpc = nt // BC  # timesteps covered per moving chunk
        with tc.tile_pool(name="pu", bufs=4, space="PSUM") as pu_pool:
            for n in range(nb // nt):
                if n == (1 if nb // nt > 1 else 0):
                    # recurrence-only loads, emitted here so they overlap
                    # the precompute instead of delaying its start
                    nc.sync.dma_start(id_sb[:], ident[:])
                    for k in range(KD):
                        nc.sync.dma_start(
                            wh_sb[:, k * H:(k + 1) * H], Wh[:, k * H:(k + 1) * H])
                # per-chunk x tiles so the first chunk's matmuls start as
                # soon as its own 8 DMAs land (not the whole 8.4MB load)
                xn = xpool.tile([128, KD * nt], bf16, tag="xn")
                for k in range(KD):
                    nc.sync.dma_start(
                        xn[:, k * nt:(k + 1) * nt],
                        xT[:, k * nb + n * nt: k * nb + (n + 1) * nt],
                    )
                for m in range(MCH):
                    ps = pu_pool.tile([128, nt], f32)
                    for k in range(KD):
                        nc.tensor.matmul(
                            ps[:],
                            wx_sb[:, k * H + m * 128: k * H + (m + 1) * 128],
                            xn[:, k * nt:(k + 1) * nt],
                            start=(k == 0),
                            stop=(k == KD - 1),
                        )
                    # psum + bias -> bf16 U tile (DVE; the ACT engine's
                    # instruction encoding only allows one sync wait on this
                    # compiler version and this op needs two)
                    nc.vector.tensor_scalar_add(
                        uv[:, n * tpc:(n + 1) * tpc, m, :],
                        ps[:],
                        b_sb[:, m:m + 1],
                    )

        # ---- Recurrence ----
        # Each step's psum group m accumulates: u_t (injected via an
        # identity-weight matmul, so no DVE add is needed) plus the 8
        # k-chunk contributions of h_{t-1} @ W_h.  PSUM-bank reads
        # serialize against ALL matmul writes to the same bank, so the
        # step's 8 groups are spread over 4 quarter tiles in separate
        # banks: the relu+bf16-cast of quarter q overlaps the matmuls of
        # quarter q+1, leaving only the last quarter's relu on the
        # serial h-chain.
        # Two 4-m-group PSUM halves per step.  Traces show the scheduler
        # interleaves the two banks' k-high batches (they are symmetric
        # in its model: both gated by relu1(t-1)), which pushes BOTH
        # stops to the stream end; the in-order DVE then fires both
        # relus at step end and the next step's k-low head eats the full
        # ~520ns stop->relu->consumer latency.  Fix: break the symmetry
        # with a REAL dependency.  relu1 is split into two DVE ops -
        # relu1a writes h_new[:,32:], relu1b writes a second copy h_hi -
        # and bank 1's k-high matmuls read h_hi.  Bank 1's k-high then
        # becomes ready ~200ns after bank 0's, so the scheduler must run
        # bank 0's k-high (and its stop) first: stop0 lands ~16 pairs
        # (~430ns) before stop1, relu0 fires ~430ns earlier, and the
        # k-low head stall shrinks to ~50-100ns.
        QSPEC = [(0, 4), (4, 4)]  # (first m, n m-groups) per PSUM half
        sched = [("id", 0), ("id", 1)]
        # k-low block: chunks 0-3 (gated by relu0(t-1))
        for k in range(4):
            for q in range(2):
                for mq in range(4):
                    sched.append(("mm", q, mq, k))
        # k-high: bank 0 reads h_new (relu1a), bank 1 reads h_hi (relu1b)
        for k in range(4, 8):
            for mq in range(4):
                sched.append(("mm", 0, mq, k))
        sched.append(("relu", 0))
        for k in range(4, 8):
            for mq in range(4):
                sched.append(("mmh", 1, mq, k))
        sched.append(("relu1a",))
        sched.append(("relu1b",))

        with tc.tile_pool(name="ph", bufs=8, space="PSUM") as ph_pool, \
                tc.tile_pool(name="hh", bufs=4) as hhpool:
            h_prev = hpool.tile([128, SW], bf16, tag="h")
            hh_prev = hhpool.tile([128, SW // 2], bf16, tag="hh")
            nc.vector.memset(h_prev[:], 0.0)
            nc.vector.memset(hh_prev[:], 0.0)
            for t in range(t_steps):
                h_new = hpool.tile([128, SW], bf16, tag="h")
                hh_new = hhpool.tile([128, SW // 2], bf16, tag="hh")
                qps = {}
                for op in sched:
                    if op[0] == "id":
                        q = op[1]
                        m0, ng = QSPEC[q]
                        qps[q] = ph_pool.tile(
                            [128, ng * BC], f32, tag="ph", name="phq")
                        nc.tensor.matmul(
                            qps[q][:],
                            id_sb[:],
                            u_sb[:, t * SW + m0 * BC: t * SW + (m0 + ng) * BC],
                            start=True,
                            stop=False,
                        )
                    elif op[0] in ("mm", "mmh"):
                        _, q, mq, k = op
                        m0, ng = QSPEC[q]
                        is_stop = (mq == ng - 1 and k == KH - 1)
                        if t == 0 and not is_stop:
                            continue  # h_0 = 0: keep only the stop marker
                        m = m0 + mq
                        if op[0] == "mmh":
                            moving = hh_prev[:, (k - 4) * BC:(k - 3) * BC]
                        else:
                            moving = h_prev[:, k * BC:(k + 1) * BC]
                        nc.tensor.matmul(
                            qps[q][:, mq * BC:(mq + 1) * BC],
                            wh_sb[:, k * H + m * 128: k * H + (m + 1) * 128],
                            moving,
                            start=False,
                            stop=is_stop,
                        )
                    elif op[0] == "relu":
                        q = op[1]
                        m0, ng = QSPEC[q]
                        nc.vector.tensor_scalar_max(
                            h_new[:, m0 * BC:(m0 + ng) * BC], qps[q][:], 0.0)
                    elif op[0] == "relu1a":
                        nc.vector.tensor_scalar_max(
                            h_new[:, SW // 2:], qps[1][:], 0.0)
                    else:  # relu1b: second copy of chunks 4-7 for bank 1
                        nc.vector.tensor_scalar_max(
                            hh_new[:], qps[1][:], 0.0)
                nc.sync.dma_start(Y[t], h_new[:])
                h_prev = h_new
                hh_prev = hh_new

    nc.compile()  # bacc passes: wait splitting, reg alloc, nop fusion, ...
    return nc


def _prep_inputs(x: np.ndarray, W: np.ndarray, b: np.ndarray, t_steps: int):
    """Host-side reshapes/casts into the per-core hidden-major layout."""
    nb = t_steps * BC
    Wx, Wh = W[:D], W[D:]
    # [d, h] -> [128, kd*H] with partition = d % 128 (within chunk)
    wx_np = np.ascontiguousarray(
        Wx.reshape(KD, 128, H).transpose(1, 0, 2).reshape(128, KD * H)
    ).astype(BF16)
    wh_np = np.ascontiguousarray(
        Wh.reshape(KH, 128, H).transpose(1, 0, 2).reshape(128, KH * H)
    ).astype(BF16)
    b_np = np.ascontiguousarray(b.reshape(MCH, 128).T).astype(np.float32)

    in_maps = []
    for c in range(NCORES):
        xc = x[c * BC:(c + 1) * BC, :t_steps]  # [BC, t, D]
        # xT[p, k*nb + t*BC + b] = xc[b, t, k*128+p]
        xt = (
            xc.transpose(2, 1, 0)              # [D, t, BC]
            .reshape(KD, 128, nb)
            .transpose(1, 0, 2)
            .reshape(128, KD * nb)
        )
        in_maps.append({
            "xT": np.ascontiguousarray(xt).astype(BF16),
            "Wx": wx_np,
            "Wh": wh_np,
            "bias": b_np,
            "ident": np.eye(128, dtype=BF16),
        })
    return in_maps


def _assemble_output(results, t_steps: int) -> np.ndarray:
    """[t, 128, SW] bf16 per core -> [B, t, H] f32."""
    y = np.empty((B, t_steps, H), dtype=np.float32)
    for c, res in enumerate(results):
        yc = np.asarray(res["Y"]).astype(np.float32)       # [t, 128, SW]
        yc = yc.reshape(t_steps, 128, MCH, BC).transpose(3, 0, 2, 1)
        y[c * BC:(c + 1) * BC] = yc.reshape(BC, t_steps, H)
    return y


def kernel(x: np.ndarray, W: np.ndarray, b: np.ndarray, **run_kwargs) -> np.ndarray:
    t_steps = x.shape[1]
    nc = build_nc(t_steps)
    in_maps = _prep_inputs(np.asarray(x), np.asarray(W), np.asarray(b), t_steps)
    res = run_bass_kernel_spmd(nc, in_maps, core_ids=list(range(NCORES)), **run_kwargs)
    out = _assemble_output(res.results, t_steps)
    if run_kwargs:
        kernel.last_result = res  # stash for profiling harnesses
    return out

